# revision 1
# baseline (speedup 1.0000x reference)
"""Trainium2 Bass kernel v4: 11x11 valid cross-correlation, 6144x6144 fp32,
SPMD 8 cores, S=12 column-phase decomposition.

Key idea vs the S=11 baseline: with 12 column phases and 10-row blocks the
contraction is K = 10*12 = 120 <= 128 and each output tile spans exactly
512 phase-blocks of columns (512*12 = 6144) -- one full PSUM bank. Every
tile needs exactly 4 matmuls (set1 s=0,1 from its own 10-row block, set2
s=0,1 from the previous block), all contiguous N=512 streams:
  308 matmuls/core (vs 564), none weight-load-bound, stationary padded to
  M=128 columns (Fast Weight Load).

Tap coverage: out col 12m+q needs input col 12(m+s)+p with v = p-q+12s in
[0,10]; s=0 covers p>=q, s=1 covers q-p in [2,12] -- complete, disjoint.
Rows: out row 10j-10+i (tile j, i in 0..9) gets u=w+10-i from block j
(w<=i) and u=w-i from block j-1 (w>=i) -- complete, disjoint.

Host side (not device-timed) shuffles into phase layout; every DMA is
contiguous per partition.
"""

import time

import numpy as np
import ml_dtypes

try:
    from concourse import bacc, mybir
except ImportError:
    import sys
    sys.path.insert(0, "/opt/trn_rl_repo")
    from concourse import bacc, mybir
import concourse.tile as tile
from concourse.bass_utils import run_bass_kernel_spmd

KH = KW = 11
H = W = 6144
OH = OW = H - (KH - 1)          # 6134

N_CORES = 8
S = 12                          # column phases
RW = 10                         # rows per block
K = RW * S                      # 120 contraction partitions
M1 = RW * S                     # 120 used output partitions
MP = 128                        # padded stationary columns (FWL)

CORE_OUT = 768                  # output rows per core
NTIL = 77                       # output tiles j=1..77 (rows 0..769)
NBLK = 78                       # input 10-row blocks per core (0..77)
ROWS_IN = NBLK * RW             # 780 input rows per core (zero-padded)
NMB = 513                       # m-positions per block slab (512 + s-shift)
NOB = 512                       # output m-blocks per tile row
GRP = 16                        # blocks per batched DMA

_prog_cache: dict = {}


def _build_program(reps: int = 1, timing: bool = False):
    key = (reps, timing)
    if key in _prog_cache:
        return _prog_cache[key]

    bf16 = mybir.dt.bfloat16
    f32 = mybir.dt.float32
    nc = bacc.Bacc("TRN2", target_bir_lowering=False, debug=False,
                   num_devices=N_CORES)

    if timing:
        xp = nc.dram_tensor("xp", [K, NBLK * NMB], bf16).ap()
        outp = nc.dram_tensor("outp", [NTIL * M1, NOB], bf16).ap()
        tout = nc.dram_tensor("tout", [NTIL, 64], bf16,
                              kind="ExternalOutput").ap()
    else:
        xp = nc.dram_tensor("xp", [K, NBLK * NMB], bf16,
                            kind="ExternalInput").ap()
        outp = nc.dram_tensor("outp", [NTIL * M1, NOB], bf16,
                              kind="ExternalOutput").ap()
    tw = nc.dram_tensor("tw", [K, 4 * MP], bf16, kind="ExternalInput").ap()

    ngrp = (NBLK + GRP - 1) // GRP          # 10 input DMA groups
    with tile.TileContext(nc) as tc:
        with (
            tc.tile_pool(name="twp", bufs=1) as twp,
            tc.tile_pool(name="xpool", bufs=4) as xpool,
            tc.tile_pool(name="pspool", bufs=8, space="PSUM") as pspool,
            tc.tile_pool(name="opool", bufs=3) as opool,
        ):
            twt = twp.tile([K, 4 * MP], bf16)
            nc.sync.dma_start(twt[:], tw[:])

            for _ in range(reps):
                xts = {}
                ot = None
                nout = 0
                for g in range(ngrp):
                    nb = min(GRP, NBLK - g * GRP)
                    xt = xpool.tile([K, GRP, NMB], bf16)
                    nc.sync.dma_start(
                        xt[:, :nb, :],
                        xp[:, g * GRP * NMB:(g * GRP + nb) * NMB].rearrange(
                            "k (b m) -> k b m", b=nb))
                    xts[g] = xt

                    # process tiles whose blocks are now resident:
                    # tile j needs blocks j-1, j  (j = 1..NTIL)
                    jlo = g * GRP + (1 if g == 0 else 0)
                    jhi = min(g * GRP + nb - 1, NTIL)
                    for j in range(jlo, jhi + 1):
                        if ot is None:
                            ot = opool.tile([K, GRP * NOB], bf16)
                            o0 = j
                        bprev, bcur = j - 1, j
                        xprev = xts[bprev // GRP]
                        xcur = xts[bcur // GRP]
                        ps = pspool.tile([MP, NOB], f32, tag="psb")
                        nc.tensor.matmul(
                            ps[:], twt[:, 2 * MP:3 * MP],
                            xprev[:, bprev % GRP, 0:NOB],
                            start=True, stop=False, skip_group_check=True)
                        nc.tensor.matmul(
                            ps[:], twt[:, 3 * MP:4 * MP],
                            xprev[:, bprev % GRP, 1:1 + NOB],
                            start=False, stop=False, skip_group_check=True)
                        nc.tensor.matmul(
                            ps[:], twt[:, 0:MP],
                            xcur[:, bcur % GRP, 0:NOB],
                            start=False, stop=False, skip_group_check=True)
                        nc.tensor.matmul(
                            ps[:], twt[:, MP:2 * MP],
                            xcur[:, bcur % GRP, 1:1 + NOB],
                            start=False, stop=True, skip_group_check=True)
                        if j % 2 == 0:
                            nc.vector.tensor_copy(
                                ot[:, nout * NOB:(nout + 1) * NOB],
                                ps[:K, :])
                        else:
                            nc.scalar.copy(
                                ot[:, nout * NOB:(nout + 1) * NOB],
                                ps[:K, :])
                        nout += 1
                        if nout == GRP or j == NTIL:
                            nc.scalar.dma_start(
                                outp[(o0 - 1) * M1:(o0 - 1 + nout) * M1,
                                     :].rearrange("(b k) m -> k b m", k=M1),
                                ot[:, :nout * NOB].rearrange(
                                    "k (b m) -> k b m", b=nout))
                            ot = None
                            nout = 0

            if timing:
                nc.sync.dma_start(tout[:, :], outp[0:NTIL * M1:M1, 0:64])

    nc.compile()
    _prog_cache[key] = nc
    return nc


def _build_tw(weight: np.ndarray) -> np.ndarray:
    """[K, 4*MP] bf16 stationary: [set1 s0 | set1 s1 | set2 s0 | set2 s1].

    set1: tile j <- block j     : u = w+10-i  (valid when w <= i)
    set2: tile j <- block j-1   : u = w-i     (valid when w >= i)
    cols: v = p-q+12s, valid 0..10.
    """
    w_ = np.asarray(weight, np.float32)
    tb1 = np.zeros((2, K, MP), np.float32)
    tb2 = np.zeros((2, K, MP), np.float32)
    for s in range(2):
        for w in range(RW):
            for p in range(S):
                for q in range(S):
                    v = p - q + S * s
                    if not (0 <= v <= KW - 1):
                        continue
                    for i in range(RW):
                        u = w + 10 - i
                        if 0 <= u <= KH - 1:
                            tb1[s, w * S + p, i * S + q] = w_[u, v]
                        u = w - i
                        if 0 <= u <= KH - 1:
                            tb2[s, w * S + p, i * S + q] = w_[u, v]
    return np.ascontiguousarray(np.concatenate(
        [tb1[0], tb1[1], tb2[0], tb2[1]], axis=1)).astype(ml_dtypes.bfloat16)


def _shard_inputs(X: np.ndarray, weight: np.ndarray):
    Xf = np.asarray(X, np.float32)
    twc = _build_tw(weight)
    in_maps = []
    for kcore in range(N_CORES):
        r0 = CORE_OUT * kcore
        xs = np.zeros((ROWS_IN, NMB * S), np.float32)   # 780 x 6156
        n = min(ROWS_IN, H - r0)
        xs[:n, :W] = Xf[r0:r0 + n]
        arr = xs.reshape(NBLK, RW, NMB, S)              # [b, r, m, p]
        xpk = np.ascontiguousarray(
            arr.transpose(1, 3, 0, 2)).reshape(K, NBLK * NMB).astype(
            ml_dtypes.bfloat16)
        in_maps.append({"xp": xpk, "tw": twc})
    return in_maps


def _assemble_output(results, bias_val: float) -> np.ndarray:
    out = np.empty((OH, OW), np.float32)
    for kcore in range(N_CORES):
        op = np.asarray(results[kcore]["outp"], np.float32).reshape(
            NTIL, RW, S, NOB)                           # [j', i, q, m]
        rows = np.ascontiguousarray(op.transpose(0, 1, 3, 2)).reshape(
            NTIL * RW, S * NOB)                         # row 10j'+i, col 12m+q
        r0 = CORE_OUT * kcore
        take = min(CORE_OUT, OH - r0)
        out[r0:r0 + take] = rows[:take, :OW]
    if bias_val != 0.0:
        out += bias_val
    return out


def kernel(X: np.ndarray, weight: np.ndarray, bias: np.ndarray) -> np.ndarray:
    nc = _build_program(reps=1)
    in_maps = _shard_inputs(X, weight)
    last_err = None
    for attempt in range(4):
        try:
            res = run_bass_kernel_spmd(nc, in_maps, list(range(N_CORES)))
            break
        except Exception as e:  # transient device wedge: wait and retry
            last_err = e
            time.sleep(90)
    else:
        raise last_err
    return _assemble_output(res.results, float(np.asarray(bias).reshape(-1)[0]))

